# revision 28
# baseline (speedup 1.0000x reference)
"""Trainium2 Bass kernel for the memristor-crossbar layer (nn_CustomLayer_30588757082254).

out = unmap(x @ G_eff) + bias, where G_eff = 1/(1/G + R_par) is an elementwise
transform of weight.T with globally min/max-normalized conductances.

Sharding: 4x2 (batch 4-way x output-column 2-way). Each core owns x rows
[b*2048,(b+1)*2048) and W columns [h*1024,(h+1)*1024).

Math (S = 1/s folds the unmapping scale; kappa*rowsum(x) + bias enter PSUM
via a K=2 fp32r init matmul, so the GEMM streams pure fp16):
  s = (g_max-g_min)/(wmax-wmin);  a = g_min/s - wmin;  kappa = wmin - g_min/s
  ge  := S*G_eff = recip(recip(W16+a) + rk[kt] + cv)     (fp32 chain, fp16 out)
  out = [kappa*xs; 1]^T [1; bias] + x16 @ ge             (xs = rowsum(x), host)

fp16 x and W halve HBM traffic (21 MB/core total) with error ~= fp32r baseline
(verified by host sim: absmax-scaled 2.9e-4). PE floor: 544 matmuls x 213 ns
= 116 us/core; everything else hides under it.

Per 512-col strip: ACT does the fp16->fp32 (+a) cast-in; DVE runs the serial
core recip -> (+rk+cv) -> recip, the last recip writing fp16 straight into
the ge tile (the fp32 constraint of the NR bit-trick is on the input only).
Single-engine chain => no cross-engine round-trips; DVE cadence ~1.8 us/strip
vs 1.7 us/strip PE consumption during the head only.

Epilogue: ACT Copy psum->sbuf (ACT is idle after the 32 cast-ins, so banks
drain at dep time) + Pool-issued DMA out. Wave order A(nb0,mb0-7),
C(nb0,mb8-15), B(nb1,mb0-7), D(nb1,mb8-15): nb1 strips aren't consumed until
~60 us, giving the transform huge slack. A/C/B run kt-major with per-group
tails on the last 4 k-tiles (bank drains stagger to match the ACT epilogue
pace); D runs group-major. Dummy matmuls warm the PE p-state before the
first strip lands.
"""
import numpy as np

import concourse.bass as bass
import concourse.mybir as mybir
import concourse.tile as tile
from concourse import bacc
from concourse.bass_utils import run_bass_kernel_spmd
from concourse.dve_ops import RECIP_APPROX_FAST_CONSTS, RECIPROCAL_APPROX_FAST

F32 = mybir.dt.float32
F32R = mybir.dt.float32r
F16 = mybir.dt.float16
AF = mybir.ActivationFunctionType
ALU = mybir.AluOpType
CRC = RECIP_APPROX_FAST_CONSTS

N_CORES = 8
B, K, N = 8192, 2048, 2048
BSH, NSH = 4, 2             # batch shards x column shards
BC = B // BSH               # 2048 batch rows per core
NC = N // NSH               # 1024 output cols per core
KT = K // 128               # 16 k-tiles
NDUMMY = 6                  # PE p-state warmup matmuls

PARASITIC_R = 2.0
G_MIN, G_MAX = 1.0 / 100000.0, 1.0 / 1000.0

_CACHE = {}


def _build_nc():
    nc = bacc.Bacc("TRN2", target_bir_lowering=False, debug=False,
                   num_devices=N_CORES)
    wt_in = nc.dram_tensor("wt", [K, NC], F16, kind="ExternalInput")
    xt_in = nc.dram_tensor("xt", [K, BC], F16, kind="ExternalInput")
    xs2_in = nc.dram_tensor("xs2", [2, BC], F32R, kind="ExternalInput")
    kb2_in = nc.dram_tensor("kb2", [2, NC], F32R, kind="ExternalInput")
    kx_in = nc.dram_tensor("kapxs", [128, 16], F32, kind="ExternalInput")
    bb_in = nc.dram_tensor("bias_bc", [128, NC], F32, kind="ExternalInput")
    mmx_in = nc.dram_tensor("mmx", [128, 18], F32, kind="ExternalInput")
    cv_in = nc.dram_tensor("cv", [128, NC], F32, kind="ExternalInput")
    out_d = nc.dram_tensor("out", [BC, NC], F32, kind="ExternalOutput")

    with tile.TileContext(nc) as tc:
        with (
            tc.tile_pool(name="xtp", bufs=1) as xtp,
            tc.tile_pool(name="wsp", bufs=10) as wsp,
            tc.tile_pool(name="tsp", bufs=28) as tsp,
            tc.tile_pool(name="gep", bufs=1) as gep,
            tc.tile_pool(name="osbp", bufs=8) as osbp,
            tc.tile_pool(name="smallp", bufs=1) as sp,
            tc.tile_pool(name="pcp", bufs=8, space="PSUM") as pcp,
        ):
            # ------------- small inputs (SP + ACT rings, early) ------------
            with nc.named_scope("setup"):
                bcv = sp.tile([128, 18], F32, tag="bcv")
                nc.scalar.dma_start(out=bcv[:], in_=mmx_in[:])
                cvt = sp.tile([128, NC], F32, tag="cv")
                nc.scalar.dma_start(out=cvt[:], in_=cv_in[:])
                xs2 = sp.tile([2, BC], F32R, tag="xs2")
                nc.sync.dma_start(out=xs2[:], in_=xs2_in[:])
                kb2 = sp.tile([2, NC], F32R, tag="kb2")
                nc.sync.dma_start(out=kb2[:], in_=kb2_in[:])
                # PE warmup fodder
                dl = sp.tile([1, 128], F16, tag="dl")
                nc.vector.memset(dl[:], 1.0)
                dr = sp.tile([1, 512], F16, tag="dr")
                nc.vector.memset(dr[:], 0.0)

            def rk(kt):
                return bcv[:, 1 + kt:2 + kt]

            # x tiles in column halves on the SP ring: first halves feed
            # waves A/B (mb0-7), second halves follow for C/D.
            xts1 = [xtp.tile([128, 1024], F16, tag=f"x1_{kt}",
                             name=f"x1_{kt}") for kt in range(KT)]
            xts2 = [xtp.tile([128, 1024], F16, tag=f"x2_{kt}",
                             name=f"x2_{kt}") for kt in range(KT)]
            for kt in range(KT):
                nc.sync.dma_start(out=xts1[kt][:],
                                  in_=xt_in[kt * 128:(kt + 1) * 128, 0:1024])
            for kt in range(KT):
                nc.sync.dma_start(out=xts2[kt][:],
                                  in_=xt_in[kt * 128:(kt + 1) * 128, 1024:2048])
            # epilogue constants ride the SP ring last (needed from ~70us)
            kx = sp.tile([128, 16], F32, tag="kx")
            nc.sync.dma_start(out=kx[:], in_=kx_in[:])
            bb = sp.tile([128, NC], F32, tag="bb")
            nc.sync.dma_start(out=bb[:], in_=bb_in[:])

            # W strips on the Pool ring, nb0 first (head-critical). The wsp
            # rotation (buf k reused by strip k+10) paces the nb1 stream
            # naturally behind the ACT cast-ins.
            wss = [wsp.tile([128, 512], F16, tag="ws", name=f"ws{i}")
                   for i in range(2 * KT)]
            for nb in range(2):
                for kt in range(KT):
                    nc.gpsimd.dma_start(
                        out=wss[nb * KT + kt][:],
                        in_=wt_in[kt * 128:(kt + 1) * 128,
                                  nb * 512:(nb + 1) * 512])

            # warmup matmuls (PE queue head; ends ~when strip0 is ready)
            with nc.named_scope("warm"):
                pcd = pcp.tile([128, 512], F32, tag="pc", name="pcd")
                for _ in range(NDUMMY):
                    nc.tensor.matmul(pcd[:], dl[:], dr[:],
                                     start=True, stop=True)

            ges = [[None] * KT, [None] * KT]

            # ---- transform: serial 3-op DVE chain per strip ----
            # W ships host-shifted as (w.T + a) in fp16, so the first recip
            # reads it directly (the NR seed's BITWISE_NOT acts on the
            # pipeline's converted fp32 value); the last recip writes fp16
            # straight into the ge tile.
            for nb in range(2):
                with nc.named_scope(f"t{nb}s"):
                    for k in range(KT):
                        ts = tsp.tile([128, 512], F32, tag="ts",
                                      name=f"ts{nb}_{k}")
                        nc.vector._custom_dve(
                            RECIPROCAL_APPROX_FAST, out=ts[:],
                            in0=wss[nb * KT + k][:],
                            s0=CRC["s0"], s1=CRC["s1"], imm2=CRC["imm2"])
                        nc.vector.scalar_tensor_tensor(
                            ts[:], ts[:], rk(k),
                            cvt[:, nb * 512:(nb + 1) * 512],
                            ALU.add, ALU.add)
                        g = gep.tile([128, 512], F16, tag=f"ge{nb}_{k}",
                                     name=f"ge{nb}_{k}")
                        nc.vector._custom_dve(
                            RECIPROCAL_APPROX_FAST, out=g[:], in0=ts[:],
                            s0=CRC["s0"], s1=CRC["s1"], imm2=CRC["imm2"])
                        ges[nb][k] = g

            # ---- epilogue: ACT psum->sbuf (+kappa*xs via per-partition
            # bias), DVE bias-row add (doesn't gate the bank drain),
            # Pool-issued DMA out. Wave A instead seeds kappa*xs+bias into
            # PSUM with a K=2 init matmul (its head is production-paced, so
            # the extra PE work is free there) and uses a plain copy.
            def epi(pc, nb, mb, tag, fold):
                osb = osbp.tile([128, 512], F32, tag="osb",
                                name=f"ep{nb}_{mb}")
                if fold:
                    nc.scalar.activation(osb[:], pc[:], AF.Identity,
                                         bias=kx[:, mb:mb + 1], scale=1.0)
                    nc.vector.tensor_tensor(
                        osb[:], osb[:], bb[:, nb * 512:(nb + 1) * 512],
                        ALU.add)
                else:
                    nc.scalar.copy(osb[:], pc[:])
                nc.gpsimd.dma_start(
                    out=out_d[mb * 128:(mb + 1) * 128,
                              nb * 512:(nb + 1) * 512],
                    in_=osb[:])

            def xs_ap(kt, mb):
                xt = xts1[kt] if mb < 8 else xts2[kt]
                j = mb if mb < 8 else mb - 8
                return xt[:, j * 128:(j + 1) * 128]

            def init_mm(pc, nb, mb):
                # pc = kappa*xs[m] + bias[n]  (rank-2 fp32r seed)
                nc.tensor.matmul(pc[:], xs2[:, mb * 128:(mb + 1) * 128],
                                 kb2[:, nb * 512:(nb + 1) * 512],
                                 start=True, stop=False)

            # kt-major over 8 banks, per-group tails on the last 4 k-tiles
            # so bank drains stagger to the ACT epilogue pace
            def head_wave(nb, mb0, tag, init):
                mbs = list(range(mb0, mb0 + 8))
                pcs = [pcp.tile([128, 512], F32, tag="pc",
                                name=f"pc_{tag}_{mb}") for mb in mbs]
                with nc.named_scope(f"mm{tag}"):
                    if init:
                        for i, mb in enumerate(mbs):
                            init_mm(pcs[i], nb, mb)
                    for kt in range(KT - 4):
                        gmov = ges[nb][kt][:]
                        for i, mb in enumerate(mbs):
                            nc.tensor.matmul(pcs[i][:], xs_ap(kt, mb), gmov,
                                             start=(not init and kt == 0),
                                             stop=False)
                    for i, mb in enumerate(mbs):
                        for kt in range(KT - 4, KT):
                            nc.tensor.matmul(pcs[i][:], xs_ap(kt, mb),
                                             ges[nb][kt][:], start=False,
                                             stop=(kt == KT - 1))
                        epi(pcs[i], nb, mb, tag, fold=not init)

            # group-major final wave (everything resident, natural stagger)
            def tail_wave(nb, mb0, tag):
                mbs = list(range(mb0, mb0 + 8))
                pcs = [pcp.tile([128, 512], F32, tag="pc",
                                name=f"pc_{tag}_{mb}") for mb in mbs]
                with nc.named_scope(f"mm{tag}"):
                    for i, mb in enumerate(mbs):
                        for kt in range(KT):
                            nc.tensor.matmul(pcs[i][:], xs_ap(kt, mb),
                                             ges[nb][kt][:],
                                             start=(kt == 0),
                                             stop=(kt == KT - 1))
                        epi(pcs[i], nb, mb, tag, fold=True)

            head_wave(0, 0, "A", init=True)
            head_wave(1, 0, "B", init=False)
            head_wave(0, 8, "C", init=False)
            tail_wave(1, 8, "D")
    nc.finalize()
    return nc


def _prep_inputs(x, weight, bias):
    wtT = np.ascontiguousarray(weight.T)          # [K, N] f32
    wmin = float(wtT.min())
    wmax = float(wtT.max())
    s = np.float32((G_MAX - G_MIN) / (wmax - wmin))
    a = np.float32(G_MIN / s - wmin)
    kappa = np.float32(wmin - G_MIN / s)
    wt16 = (wtT + a).astype(np.float16)           # host-shifted W' = w.T + a

    mmx = np.zeros((128, 18), dtype=np.float32)
    mmx[:, 0] = a
    mmx[:, 1:17] = [-256.0 * kt * s for kt in range(KT)]
    mmx[:, 17] = -2.0 * s * np.arange(128, dtype=np.float64)

    # closed-form parasitic term, pre-scaled by s:
    #   u = recip(w+a) + rk[kt] + cv,  rk[kt] = -256*kt*s,
    #   cv[p, j] = s*(4098 + 2*n_abs - 2*p), n_abs = h*NC + j
    parange = np.arange(128, dtype=np.float64)
    narange = np.arange(N, dtype=np.float64)
    cv_row = (np.float64(s) * (4098.0 + 2.0 * narange[None, :]
                               - 2.0 * parange[:, None])).astype(np.float32)

    xs = x.astype(np.float64).sum(axis=1).astype(np.float32)  # [B]
    bias2 = bias.astype(np.float32)
    in_maps = []
    for c in range(N_CORES):
        b, h = divmod(c, NSH)
        x_c = x[b * BC:(b + 1) * BC, :]           # [BC, K]
        xt_c = np.ascontiguousarray(x_c.T.astype(np.float16))   # [K, BC]
        wt_c = np.ascontiguousarray(wt16[:, h * NC:(h + 1) * NC])
        cv_c = np.ascontiguousarray(cv_row[:, h * NC:(h + 1) * NC])
        xs2_c = np.empty((2, BC), dtype=np.float32)
        xs2_c[0] = kappa * xs[b * BC:(b + 1) * BC]
        xs2_c[1] = 1.0
        kb2_c = np.empty((2, NC), dtype=np.float32)
        kb2_c[0] = 1.0
        kb2_c[1] = bias2[h * NC:(h + 1) * NC]
        kapxs_c = np.ascontiguousarray(xs2_c[0].reshape(16, 128).T)
        bb_c = np.ascontiguousarray(np.broadcast_to(
            bias2[h * NC:(h + 1) * NC][None, :], (128, NC)))
        in_maps.append({"wt": wt_c, "xt": xt_c, "xs2": xs2_c, "kb2": kb2_c,
                        "kapxs": kapxs_c, "bias_bc": bb_c,
                        "mmx": mmx, "cv": cv_c})
    return in_maps


def _run(x, weight, bias, trace=False, trace_kwargs=None):
    if "nc" not in _CACHE:
        _CACHE["nc"] = _build_nc()
    nc = _CACHE["nc"]
    in_maps = _prep_inputs(x, weight, bias)
    res = run_bass_kernel_spmd(nc, in_maps, list(range(N_CORES)), trace=trace,
                               **(trace_kwargs or {}))
    out = np.empty((B, N), dtype=np.float32)
    for c in range(N_CORES):
        b, h = divmod(c, NSH)
        out[b * BC:(b + 1) * BC, h * NC:(h + 1) * NC] = res.results[c]["out"]
    return out, res


def kernel(x, weight, bias):
    x = np.asarray(x, dtype=np.float32)
    weight = np.asarray(weight, dtype=np.float32)
    bias = np.asarray(bias, dtype=np.float32)
    out, _ = _run(x, weight, bias, trace=False)
    return out.astype(np.float32)


# revision 29
# speedup vs baseline: 1.0033x; 1.0033x over previous
"""Trainium2 Bass kernel for the memristor-crossbar layer (nn_CustomLayer_30588757082254).

out = unmap(x @ G_eff) + bias, where G_eff = 1/(1/G + R_par) is an elementwise
transform of weight.T with globally min/max-normalized conductances.

Sharding: 4x2 (batch 4-way x output-column 2-way). Each core owns x rows
[b*2048,(b+1)*2048) and W columns [h*1024,(h+1)*1024).

Math (S = 1/s folds the unmapping scale; W ships host-shifted as
W' = w.T + a in fp16):
  s = (g_max-g_min)/(wmax-wmin);  a = g_min/s - wmin;  kappa = wmin - g_min/s
  ge  := S*G_eff = recip(recip(W'16) + rk[kt] + cv)      (fp32 chain, fp16 out)
  out = x16 @ ge + kappa*xs + bias                       (xs = rowsum(x), host)

fp16 x and W halve HBM traffic (21 MB/core total) with error ~= fp32r baseline
(measured absmax-scaled 4.5e-4 vs baseline's 2.8e-4, gate 2e-2). PE floor:
520 matmuls x 216 ns ~= 112 us/core; everything else hides under it.

Per 512-col strip the transform is a serial 3-op DVE chain: recip (reading
the fp16 W' tile directly — the NR bit-trick constrains only the pipeline
value, which is already converted to fp32) -> +rk+cv stt -> recip writing
fp16 straight into the ge tile. Single-engine chain => no cross-engine
round-trips; ~1.9 us/strip vs 1.73 us/strip PE consumption during the head.

Waves: A(nb0,mb0-7), B(nb1,mb0-7), C(nb0,mb8-15) kt-major with per-group
tails on the last 4 k-tiles (bank drains stagger to the ACT epilogue pace);
D(nb1,mb8-15) group-major. Wave A seeds kappa*xs+bias into PSUM with a K=2
fp32r init matmul (its head is transform-paced, so the PE work is free) and
drains with a plain ACT copy; B/C/D skip the init and instead fold kappa*xs
into the ACT drain (per-partition bias) and the bias row into a DVE
tensor_tensor add that gates only the DMA out, not the bank. Dummy matmuls
warm the PE p-state before the first strip lands; DMA rings: SP carries x,
Pool carries W + outs, ACT carries the small constants.
"""
import numpy as np

import concourse.bass as bass
import concourse.mybir as mybir
import concourse.tile as tile
from concourse import bacc
from concourse.bass_utils import run_bass_kernel_spmd
from concourse.dve_ops import RECIP_APPROX_FAST_CONSTS, RECIPROCAL_APPROX_FAST

F32 = mybir.dt.float32
F32R = mybir.dt.float32r
F16 = mybir.dt.float16
AF = mybir.ActivationFunctionType
ALU = mybir.AluOpType
CRC = RECIP_APPROX_FAST_CONSTS

N_CORES = 8
B, K, N = 8192, 2048, 2048
BSH, NSH = 4, 2             # batch shards x column shards
BC = B // BSH               # 2048 batch rows per core
NC = N // NSH               # 1024 output cols per core
KT = K // 128               # 16 k-tiles
NDUMMY = 6                  # PE p-state warmup matmuls

PARASITIC_R = 2.0
G_MIN, G_MAX = 1.0 / 100000.0, 1.0 / 1000.0

_CACHE = {}


def _build_nc():
    nc = bacc.Bacc("TRN2", target_bir_lowering=False, debug=False,
                   num_devices=N_CORES)
    wt_in = nc.dram_tensor("wt", [K, NC], F16, kind="ExternalInput")
    xt_in = nc.dram_tensor("xt", [K, BC], F16, kind="ExternalInput")
    xs2_in = nc.dram_tensor("xs2", [2, BC], F32R, kind="ExternalInput")
    kb2_in = nc.dram_tensor("kb2", [2, NC], F32R, kind="ExternalInput")
    kx_in = nc.dram_tensor("kapxs", [128, 16], F32, kind="ExternalInput")
    bb_in = nc.dram_tensor("bias_bc", [128, NC], F32, kind="ExternalInput")
    mmx_in = nc.dram_tensor("mmx", [128, 18], F32, kind="ExternalInput")
    cv_in = nc.dram_tensor("cv", [128, NC], F32, kind="ExternalInput")
    out_d = nc.dram_tensor("out", [BC, NC], F32, kind="ExternalOutput")

    with tile.TileContext(nc) as tc:
        with (
            tc.tile_pool(name="xtp", bufs=1) as xtp,
            tc.tile_pool(name="wsp", bufs=10) as wsp,
            tc.tile_pool(name="tsp", bufs=28) as tsp,
            tc.tile_pool(name="gep", bufs=1) as gep,
            tc.tile_pool(name="osbp", bufs=8) as osbp,
            tc.tile_pool(name="smallp", bufs=1) as sp,
            tc.tile_pool(name="pcp", bufs=8, space="PSUM") as pcp,
        ):
            # ------------- small inputs (SP + ACT rings, early) ------------
            with nc.named_scope("setup"):
                bcv = sp.tile([128, 18], F32, tag="bcv")
                nc.scalar.dma_start(out=bcv[:], in_=mmx_in[:])
                cvt = sp.tile([128, NC], F32, tag="cv")
                nc.scalar.dma_start(out=cvt[:], in_=cv_in[:])
                xs2 = sp.tile([2, BC], F32R, tag="xs2")
                nc.sync.dma_start(out=xs2[:], in_=xs2_in[:])
                kb2 = sp.tile([2, NC], F32R, tag="kb2")
                nc.sync.dma_start(out=kb2[:], in_=kb2_in[:])
                # PE warmup fodder
                dl = sp.tile([1, 128], F16, tag="dl")
                nc.vector.memset(dl[:], 1.0)
                dr = sp.tile([1, 512], F16, tag="dr")
                nc.vector.memset(dr[:], 0.0)

            def rk(kt):
                return bcv[:, 1 + kt:2 + kt]

            # x tiles in column halves on the SP ring: first halves feed
            # waves A/B (mb0-7), second halves follow for C/D.
            xts1 = [xtp.tile([128, 1024], F16, tag=f"x1_{kt}",
                             name=f"x1_{kt}") for kt in range(KT)]
            xts2 = [xtp.tile([128, 1024], F16, tag=f"x2_{kt}",
                             name=f"x2_{kt}") for kt in range(KT)]
            for kt in range(KT):
                nc.sync.dma_start(out=xts1[kt][:],
                                  in_=xt_in[kt * 128:(kt + 1) * 128, 0:1024])
            for kt in range(KT):
                nc.sync.dma_start(out=xts2[kt][:],
                                  in_=xt_in[kt * 128:(kt + 1) * 128, 1024:2048])
            # epilogue constants ride the SP ring last (needed from ~70us)
            kx = sp.tile([128, 16], F32, tag="kx")
            nc.sync.dma_start(out=kx[:], in_=kx_in[:])
            bb = sp.tile([128, NC], F32, tag="bb")
            nc.sync.dma_start(out=bb[:], in_=bb_in[:])

            # W strips on the Pool ring, nb0 first (head-critical). The wsp
            # rotation (buf k reused by strip k+10) paces the nb1 stream
            # naturally behind the ACT cast-ins.
            wss = [wsp.tile([128, 512], F16, tag="ws", name=f"ws{i}")
                   for i in range(2 * KT)]
            for nb in range(2):
                for kt in range(KT):
                    nc.gpsimd.dma_start(
                        out=wss[nb * KT + kt][:],
                        in_=wt_in[kt * 128:(kt + 1) * 128,
                                  nb * 512:(nb + 1) * 512])

            # warmup matmuls (PE queue head; ends ~when strip0 is ready)
            with nc.named_scope("warm"):
                pcd = pcp.tile([128, 512], F32, tag="pc", name="pcd")
                for _ in range(NDUMMY):
                    nc.tensor.matmul(pcd[:], dl[:], dr[:],
                                     start=True, stop=True)

            ges = [[None] * KT, [None] * KT]

            # ---- transform: serial 3-op DVE chain per strip ----
            # W ships host-shifted as (w.T + a) in fp16, so the first recip
            # reads it directly (the NR seed's BITWISE_NOT acts on the
            # pipeline's converted fp32 value); the last recip writes fp16
            # straight into the ge tile.
            for nb in range(2):
                with nc.named_scope(f"t{nb}s"):
                    for k in range(KT):
                        ts = tsp.tile([128, 512], F32, tag="ts",
                                      name=f"ts{nb}_{k}")
                        nc.vector._custom_dve(
                            RECIPROCAL_APPROX_FAST, out=ts[:],
                            in0=wss[nb * KT + k][:],
                            s0=CRC["s0"], s1=CRC["s1"], imm2=CRC["imm2"])
                        nc.vector.scalar_tensor_tensor(
                            ts[:], ts[:], rk(k),
                            cvt[:, nb * 512:(nb + 1) * 512],
                            ALU.add, ALU.add)
                        g = gep.tile([128, 512], F16, tag=f"ge{nb}_{k}",
                                     name=f"ge{nb}_{k}")
                        nc.vector._custom_dve(
                            RECIPROCAL_APPROX_FAST, out=g[:], in0=ts[:],
                            s0=CRC["s0"], s1=CRC["s1"], imm2=CRC["imm2"])
                        ges[nb][k] = g

            # ---- epilogue: ACT psum->sbuf (+kappa*xs via per-partition
            # bias), DVE bias-row add (doesn't gate the bank drain),
            # Pool-issued DMA out. Wave A instead seeds kappa*xs+bias into
            # PSUM with a K=2 init matmul (its head is production-paced, so
            # the extra PE work is free there) and uses a plain copy.
            def epi(pc, nb, mb, tag, fold):
                osb = osbp.tile([128, 512], F32, tag="osb",
                                name=f"ep{nb}_{mb}")
                if fold:
                    nc.scalar.activation(osb[:], pc[:], AF.Identity,
                                         bias=kx[:, mb:mb + 1], scale=1.0)
                    nc.vector.tensor_tensor(
                        osb[:], osb[:], bb[:, nb * 512:(nb + 1) * 512],
                        ALU.add)
                else:
                    nc.scalar.copy(osb[:], pc[:])
                nc.gpsimd.dma_start(
                    out=out_d[mb * 128:(mb + 1) * 128,
                              nb * 512:(nb + 1) * 512],
                    in_=osb[:])

            def xs_ap(kt, mb):
                xt = xts1[kt] if mb < 8 else xts2[kt]
                j = mb if mb < 8 else mb - 8
                return xt[:, j * 128:(j + 1) * 128]

            def init_mm(pc, nb, mb):
                # pc = kappa*xs[m] + bias[n]  (rank-2 fp32r seed)
                nc.tensor.matmul(pc[:], xs2[:, mb * 128:(mb + 1) * 128],
                                 kb2[:, nb * 512:(nb + 1) * 512],
                                 start=True, stop=False)

            # kt-major over 8 banks, per-group tails on the last 4 k-tiles
            # so bank drains stagger to the ACT epilogue pace
            def head_wave(nb, mb0, tag, init):
                mbs = list(range(mb0, mb0 + 8))
                pcs = [pcp.tile([128, 512], F32, tag="pc",
                                name=f"pc_{tag}_{mb}") for mb in mbs]
                with nc.named_scope(f"mm{tag}"):
                    if init:
                        for i, mb in enumerate(mbs):
                            init_mm(pcs[i], nb, mb)
                    for kt in range(KT - 4):
                        gmov = ges[nb][kt][:]
                        for i, mb in enumerate(mbs):
                            nc.tensor.matmul(pcs[i][:], xs_ap(kt, mb), gmov,
                                             start=(not init and kt == 0),
                                             stop=False)
                    for i, mb in enumerate(mbs):
                        for kt in range(KT - 4, KT):
                            nc.tensor.matmul(pcs[i][:], xs_ap(kt, mb),
                                             ges[nb][kt][:], start=False,
                                             stop=(kt == KT - 1))
                        epi(pcs[i], nb, mb, tag, fold=not init)

            # group-major final wave (everything resident, natural stagger)
            def tail_wave(nb, mb0, tag):
                mbs = list(range(mb0, mb0 + 8))
                pcs = [pcp.tile([128, 512], F32, tag="pc",
                                name=f"pc_{tag}_{mb}") for mb in mbs]
                with nc.named_scope(f"mm{tag}"):
                    for i, mb in enumerate(mbs):
                        for kt in range(KT):
                            nc.tensor.matmul(pcs[i][:], xs_ap(kt, mb),
                                             ges[nb][kt][:],
                                             start=(kt == 0),
                                             stop=(kt == KT - 1))
                        epi(pcs[i], nb, mb, tag, fold=True)

            head_wave(0, 0, "A", init=True)
            head_wave(1, 0, "B", init=False)
            head_wave(0, 8, "C", init=False)
            tail_wave(1, 8, "D")
    nc.finalize()
    return nc


def _prep_inputs(x, weight, bias):
    wtT = np.ascontiguousarray(weight.T)          # [K, N] f32
    wmin = float(wtT.min())
    wmax = float(wtT.max())
    s = np.float32((G_MAX - G_MIN) / (wmax - wmin))
    a = np.float32(G_MIN / s - wmin)
    kappa = np.float32(wmin - G_MIN / s)
    wt16 = (wtT + a).astype(np.float16)           # host-shifted W' = w.T + a

    mmx = np.zeros((128, 18), dtype=np.float32)
    mmx[:, 0] = a
    mmx[:, 1:17] = [-256.0 * kt * s for kt in range(KT)]
    mmx[:, 17] = -2.0 * s * np.arange(128, dtype=np.float64)

    # closed-form parasitic term, pre-scaled by s:
    #   u = recip(w+a) + rk[kt] + cv,  rk[kt] = -256*kt*s,
    #   cv[p, j] = s*(4098 + 2*n_abs - 2*p), n_abs = h*NC + j
    parange = np.arange(128, dtype=np.float64)
    narange = np.arange(N, dtype=np.float64)
    cv_row = (np.float64(s) * (4098.0 + 2.0 * narange[None, :]
                               - 2.0 * parange[:, None])).astype(np.float32)

    xs = x.astype(np.float64).sum(axis=1).astype(np.float32)  # [B]
    bias2 = bias.astype(np.float32)
    in_maps = []
    for c in range(N_CORES):
        b, h = divmod(c, NSH)
        x_c = x[b * BC:(b + 1) * BC, :]           # [BC, K]
        xt_c = np.ascontiguousarray(x_c.T.astype(np.float16))   # [K, BC]
        wt_c = np.ascontiguousarray(wt16[:, h * NC:(h + 1) * NC])
        cv_c = np.ascontiguousarray(cv_row[:, h * NC:(h + 1) * NC])
        xs2_c = np.empty((2, BC), dtype=np.float32)
        xs2_c[0] = kappa * xs[b * BC:(b + 1) * BC]
        xs2_c[1] = 1.0
        kb2_c = np.empty((2, NC), dtype=np.float32)
        kb2_c[0] = 1.0
        kb2_c[1] = bias2[h * NC:(h + 1) * NC]
        kapxs_c = np.ascontiguousarray(xs2_c[0].reshape(16, 128).T)
        bb_c = np.ascontiguousarray(np.broadcast_to(
            bias2[h * NC:(h + 1) * NC][None, :], (128, NC)))
        in_maps.append({"wt": wt_c, "xt": xt_c, "xs2": xs2_c, "kb2": kb2_c,
                        "kapxs": kapxs_c, "bias_bc": bb_c,
                        "mmx": mmx, "cv": cv_c})
    return in_maps


def _run(x, weight, bias, trace=False, trace_kwargs=None):
    if "nc" not in _CACHE:
        _CACHE["nc"] = _build_nc()
    nc = _CACHE["nc"]
    in_maps = _prep_inputs(x, weight, bias)
    res = run_bass_kernel_spmd(nc, in_maps, list(range(N_CORES)), trace=trace,
                               **(trace_kwargs or {}))
    out = np.empty((B, N), dtype=np.float32)
    for c in range(N_CORES):
        b, h = divmod(c, NSH)
        out[b * BC:(b + 1) * BC, h * NC:(h + 1) * NC] = res.results[c]["out"]
    return out, res


def kernel(x, weight, bias):
    x = np.asarray(x, dtype=np.float32)
    weight = np.asarray(weight, dtype=np.float32)
    bias = np.asarray(bias, dtype=np.float32)
    out, _ = _run(x, weight, bias, trace=False)
    return out.astype(np.float32)


# revision 36
# speedup vs baseline: 1.2052x; 1.2012x over previous
"""Trainium2 Bass kernel for the memristor-crossbar layer (nn_CustomLayer_30588757082254).

out = unmap(x @ G_eff) + bias, where G_eff = 1/(1/G + R_par) is an elementwise
transform of weight.T with globally min/max-normalized conductances.

Sharding: 4x2 (batch 4-way x output-column 2-way). Each core owns x rows
[b*2048,(b+1)*2048) and W columns [h*1024,(h+1)*1024).

Math (S = 1/s folds the unmapping scale; W ships host-shifted as
W' = w.T + a in fp16):
  s = (g_max-g_min)/(wmax-wmin);  a = g_min/s - wmin;  kappa = wmin - g_min/s
  ge  := S*G_eff = recip(recip(W'16) + rk[kt] + cv)      (fp32 chain, fp16 out)
  out = x16 @ ge + kappa*xs + bias                       (xs = rowsum(x), host)

fp16 x and W halve HBM traffic (21 MB/core total) with error ~= fp32r baseline
(measured absmax-scaled 4.5e-4 vs baseline's 2.8e-4, gate 2e-2). PE floor:
520 matmuls x 216 ns ~= 112 us/core; everything else hides under it.

Per 512-col strip the transform is a serial 3-op DVE chain: recip (reading
the fp16 W' tile directly — the NR bit-trick constrains only the pipeline
value, which is already converted to fp32) -> +rk+cv stt -> recip writing
fp16 straight into the ge tile. Single-engine chain => no cross-engine
round-trips; ~1.9 us/strip vs 1.73 us/strip PE consumption during the head.

Waves: A(nb0,mb0-7), B(nb1,mb0-7), C(nb0,mb8-15) kt-major with per-group
tails on the last 4 k-tiles (bank drains stagger to the ACT epilogue pace);
D(nb1,mb8-15) group-major. Wave A seeds kappa*xs+bias into PSUM with a K=2
fp32r init matmul (its head is transform-paced, so the PE work is free) and
drains with a plain ACT copy; B/C/D skip the init and instead fold kappa*xs
into the ACT drain (per-partition bias) and the bias row into a DVE
tensor_tensor add that gates only the DMA out, not the bank. Dummy matmuls
warm the PE p-state before the first strip lands; DMA rings: SP carries x,
Pool carries W + outs, ACT carries the small constants.
"""
import numpy as np

import concourse.bass as bass
import concourse.mybir as mybir
import concourse.tile as tile
from concourse import bacc
from concourse.bass_utils import run_bass_kernel_spmd
from concourse.dve_ops import RECIP_APPROX_FAST_CONSTS, RECIPROCAL_APPROX_FAST

F32 = mybir.dt.float32
F32R = mybir.dt.float32r
F16 = mybir.dt.float16
AF = mybir.ActivationFunctionType
ALU = mybir.AluOpType
CRC = RECIP_APPROX_FAST_CONSTS

N_CORES = 8
B, K, N = 8192, 2048, 2048
BSH, NSH = 4, 2             # batch shards x column shards
BC = B // BSH               # 2048 batch rows per core
NC = N // NSH               # 1024 output cols per core
KT = K // 128               # 16 k-tiles
NDUMMY = 6                  # PE p-state warmup matmuls

PARASITIC_R = 2.0
G_MIN, G_MAX = 1.0 / 100000.0, 1.0 / 1000.0

_CACHE = {}


def _build_nc():
    nc = bacc.Bacc("TRN2", target_bir_lowering=False, debug=False,
                   num_devices=N_CORES)
    wt_in = nc.dram_tensor("wt", [K, NC], F16, kind="ExternalInput")
    xt_in = nc.dram_tensor("xt", [K, BC], F16, kind="ExternalInput")
    xs2_in = nc.dram_tensor("xs2", [2, BC], F32R, kind="ExternalInput")
    kb2_in = nc.dram_tensor("kb2", [2, NC], F32R, kind="ExternalInput")
    kx_in = nc.dram_tensor("kapxs", [128, 16], F32, kind="ExternalInput")
    bb_in = nc.dram_tensor("bias_bc", [128, NC], F32, kind="ExternalInput")
    mmx_in = nc.dram_tensor("mmx", [128, 18], F32, kind="ExternalInput")
    cv_in = nc.dram_tensor("cv", [128, NC], F32, kind="ExternalInput")
    out_d = nc.dram_tensor("out", [BC, NC], F32, kind="ExternalOutput")

    with tile.TileContext(nc) as tc:
        with (
            tc.tile_pool(name="xtp", bufs=1) as xtp,
            tc.tile_pool(name="wsp", bufs=6) as wsp,
            tc.tile_pool(name="tsp", bufs=7) as tsp,
            tc.tile_pool(name="gep", bufs=1) as gep,
            tc.tile_pool(name="osbp", bufs=8) as osbp,
            tc.tile_pool(name="smallp", bufs=1) as sp,
            tc.tile_pool(name="pcp", bufs=8, space="PSUM") as pcp,
        ):
            # ------------- small inputs (SP + ACT rings, early) ------------
            with nc.named_scope("setup"):
                bcv = sp.tile([128, 18], F32, tag="bcv")
                nc.scalar.dma_start(out=bcv[:], in_=mmx_in[:])
                cvt = sp.tile([128, NC], F32, tag="cv")
                nc.scalar.dma_start(out=cvt[:], in_=cv_in[:])
                xs2 = sp.tile([2, BC], F32R, tag="xs2")
                nc.sync.dma_start(out=xs2[:], in_=xs2_in[:])
                kb2 = sp.tile([2, NC], F32R, tag="kb2")
                nc.sync.dma_start(out=kb2[:], in_=kb2_in[:])
                # PE warmup fodder (memsets on Pool so the DVE queue opens
                # directly with the transform chain)
                dl = sp.tile([1, 128], F16, tag="dl")
                nc.gpsimd.memset(dl[:], 1.0)
                dr = sp.tile([1, 512], F16, tag="dr")
                nc.gpsimd.memset(dr[:], 0.0)

            def rk(kt):
                return bcv[:, 1 + kt:2 + kt]

            # x tiles in column halves on the SP ring: first halves feed
            # waves A/B (mb0-7), second halves follow for C/D.
            xts1 = [xtp.tile([128, 1024], F16, tag=f"x1_{kt}",
                             name=f"x1_{kt}") for kt in range(KT)]
            xts2 = [xtp.tile([128, 1024], F16, tag=f"x2_{kt}",
                             name=f"x2_{kt}") for kt in range(KT)]
            for kt in range(KT):
                nc.sync.dma_start(out=xts1[kt][:],
                                  in_=xt_in[kt * 128:(kt + 1) * 128, 0:1024])
            for kt in range(KT):
                nc.sync.dma_start(out=xts2[kt][:],
                                  in_=xt_in[kt * 128:(kt + 1) * 128, 1024:2048])
            # epilogue constants ride the SP ring last (needed from ~70us)
            kx = sp.tile([128, 16], F32, tag="kx")
            nc.sync.dma_start(out=kx[:], in_=kx_in[:])
            bb = sp.tile([128, NC], F32, tag="bb")
            nc.sync.dma_start(out=bb[:], in_=bb_in[:])

            # W arrives as host-packed strip PAIRS: dram row block
            # (nb*8+p) holds strips kt=2p,2p+1 of column-half nb side by
            # side — one 256 KB DMA per pair with contiguous 2 KB lines.
            # nb0 pairs first (head-critical).
            wss = [wsp.tile([128, 1024], F16, tag="ws", name=f"ws{i}")
                   for i in range(KT)]
            for i in range(KT):
                nc.gpsimd.dma_start(
                    out=wss[i][:],
                    in_=wt_in[i * 128:(i + 1) * 128, :])

            # warmup matmuls (PE queue head; ends ~when strip0 is ready)
            with nc.named_scope("warm"):
                pcd = pcp.tile([128, 512], F32, tag="pc", name="pcd")
                for _ in range(NDUMMY):
                    nc.tensor.matmul(pcd[:], dl[:], dr[:],
                                     start=True, stop=True)

            gep_pairs = [[None] * (KT // 2), [None] * (KT // 2)]

            # ---- transform: DVE chain over strip PAIRS ----
            # W ships host-shifted as (w.T + a) in fp16; the recips share
            # their Chebyshev/NR constants across strips, so each runs
            # [128,1024]-wide over a pair (amortizing per-op overhead) and
            # the last one writes fp16 straight into the ge pair tile. Only
            # the +rk+cv adds stay per-strip (rk is per-kt).
            for nb in range(2):
                with nc.named_scope(f"t{nb}s"):
                    for p in range(KT // 2):
                        ts = tsp.tile([128, 1024], F32, tag="ts",
                                      name=f"ts{nb}_{p}")
                        nc.vector._custom_dve(
                            RECIPROCAL_APPROX_FAST, out=ts[:],
                            in0=wss[nb * 8 + p][:],
                            s0=CRC["s0"], s1=CRC["s1"], imm2=CRC["imm2"])
                        for e in range(2):
                            nc.vector.scalar_tensor_tensor(
                                ts[:, e * 512:(e + 1) * 512],
                                ts[:, e * 512:(e + 1) * 512], rk(2 * p + e),
                                cvt[:, nb * 512:(nb + 1) * 512],
                                ALU.add, ALU.add)
                        g = gep.tile([128, 1024], F16, tag=f"ge{nb}_{p}",
                                     name=f"ge{nb}_{p}")
                        nc.vector._custom_dve(
                            RECIPROCAL_APPROX_FAST, out=g[:], in0=ts[:],
                            s0=CRC["s0"], s1=CRC["s1"], imm2=CRC["imm2"])
                        gep_pairs[nb][p] = g

            def ge_ap(nb, kt):
                return gep_pairs[nb][kt // 2][:, (kt % 2) * 512:
                                              (kt % 2 + 1) * 512]

            # ---- epilogue: ACT psum->sbuf (+kappa*xs via per-partition
            # bias), DVE bias-row add (doesn't gate the bank drain),
            # Pool-issued DMA out. Wave A instead seeds kappa*xs+bias into
            # PSUM with a K=2 init matmul (its head is production-paced, so
            # the extra PE work is free there) and uses a plain copy.
            def epi(pc, nb, mb, tag, fold):
                osb = osbp.tile([128, 512], F32, tag="osb",
                                name=f"ep{nb}_{mb}")
                if fold:
                    nc.scalar.activation(osb[:], pc[:], AF.Identity,
                                         bias=kx[:, mb:mb + 1], scale=1.0)
                    nc.vector.tensor_tensor(
                        osb[:], osb[:], bb[:, nb * 512:(nb + 1) * 512],
                        ALU.add)
                else:
                    nc.scalar.copy(osb[:], pc[:])
                nc.gpsimd.dma_start(
                    out=out_d[mb * 128:(mb + 1) * 128,
                              nb * 512:(nb + 1) * 512],
                    in_=osb[:])

            def xs_ap(kt, mb):
                xt = xts1[kt] if mb < 8 else xts2[kt]
                j = mb if mb < 8 else mb - 8
                return xt[:, j * 128:(j + 1) * 128]

            def init_mm(pc, nb, mb):
                # pc = kappa*xs[m] + bias[n]  (rank-2 fp32r seed)
                nc.tensor.matmul(pc[:], xs2[:, mb * 128:(mb + 1) * 128],
                                 kb2[:, nb * 512:(nb + 1) * 512],
                                 start=True, stop=False)

            # kt-major over 8 banks, per-group tails on the last 4 k-tiles
            # so bank drains stagger to the ACT epilogue pace
            def head_wave(nb, mb0, tag, init):
                mbs = list(range(mb0, mb0 + 8))
                pcs = [pcp.tile([128, 512], F32, tag="pc",
                                name=f"pc_{tag}_{mb}") for mb in mbs]
                with nc.named_scope(f"mm{tag}"):
                    if init:
                        for i, mb in enumerate(mbs):
                            init_mm(pcs[i], nb, mb)
                    for kt in range(KT - 4):
                        gmov = ge_ap(nb, kt)
                        for i, mb in enumerate(mbs):
                            nc.tensor.matmul(pcs[i][:], xs_ap(kt, mb), gmov,
                                             start=(not init and kt == 0),
                                             stop=False)
                    for i, mb in enumerate(mbs):
                        for kt in range(KT - 4, KT):
                            nc.tensor.matmul(pcs[i][:], xs_ap(kt, mb),
                                             ge_ap(nb, kt), start=False,
                                             stop=(kt == KT - 1))
                        epi(pcs[i], nb, mb, tag, fold=not init)

            # group-major final wave (everything resident, natural stagger)
            def tail_wave(nb, mb0, tag):
                mbs = list(range(mb0, mb0 + 8))
                pcs = [pcp.tile([128, 512], F32, tag="pc",
                                name=f"pc_{tag}_{mb}") for mb in mbs]
                with nc.named_scope(f"mm{tag}"):
                    for i, mb in enumerate(mbs):
                        for kt in range(KT):
                            nc.tensor.matmul(pcs[i][:], xs_ap(kt, mb),
                                             ge_ap(nb, kt),
                                             start=(kt == 0),
                                             stop=(kt == KT - 1))
                        epi(pcs[i], nb, mb, tag, fold=True)

            head_wave(0, 0, "A", init=True)
            head_wave(1, 0, "B", init=False)
            head_wave(0, 8, "C", init=False)
            tail_wave(1, 8, "D")
    nc.finalize()
    return nc


def _prep_inputs(x, weight, bias):
    wtT = np.ascontiguousarray(weight.T)          # [K, N] f32
    wmin = float(wtT.min())
    wmax = float(wtT.max())
    s = np.float32((G_MAX - G_MIN) / (wmax - wmin))
    a = np.float32(G_MIN / s - wmin)
    kappa = np.float32(wmin - G_MIN / s)
    wt16 = (wtT + a).astype(np.float16)           # host-shifted W' = w.T + a

    mmx = np.zeros((128, 18), dtype=np.float32)
    mmx[:, 0] = a
    mmx[:, 1:17] = [-256.0 * kt * s for kt in range(KT)]
    mmx[:, 17] = -2.0 * s * np.arange(128, dtype=np.float64)

    # closed-form parasitic term, pre-scaled by s:
    #   u = recip(w+a) + rk[kt] + cv,  rk[kt] = -256*kt*s,
    #   cv[p, j] = s*(4098 + 2*n_abs - 2*p), n_abs = h*NC + j
    parange = np.arange(128, dtype=np.float64)
    narange = np.arange(N, dtype=np.float64)
    cv_row = (np.float64(s) * (4098.0 + 2.0 * narange[None, :]
                               - 2.0 * parange[:, None])).astype(np.float32)

    xs = x.astype(np.float64).sum(axis=1).astype(np.float32)  # [B]
    bias2 = bias.astype(np.float32)
    in_maps = []
    for c in range(N_CORES):
        b, h = divmod(c, NSH)
        x_c = x[b * BC:(b + 1) * BC, :]           # [BC, K]
        xt_c = np.ascontiguousarray(x_c.T.astype(np.float16))   # [K, BC]
        # pack strip pairs: dram row block (nb*8+p) = strips kt=2p,2p+1 of
        # column-half nb side by side -> [2*8*128, 1024]
        wt_h = wt16[:, h * NC:(h + 1) * NC]       # [K, NC]
        w5 = wt_h.reshape(8, 2, 128, 2, 512)      # [p, e, i, nb, 512]
        wt_c = np.ascontiguousarray(
            w5.transpose(3, 0, 2, 1, 4).reshape(K, NC))
        cv_c = np.ascontiguousarray(cv_row[:, h * NC:(h + 1) * NC])
        xs2_c = np.empty((2, BC), dtype=np.float32)
        xs2_c[0] = kappa * xs[b * BC:(b + 1) * BC]
        xs2_c[1] = 1.0
        kb2_c = np.empty((2, NC), dtype=np.float32)
        kb2_c[0] = 1.0
        kb2_c[1] = bias2[h * NC:(h + 1) * NC]
        kapxs_c = np.ascontiguousarray(xs2_c[0].reshape(16, 128).T)
        bb_c = np.ascontiguousarray(np.broadcast_to(
            bias2[h * NC:(h + 1) * NC][None, :], (128, NC)))
        in_maps.append({"wt": wt_c, "xt": xt_c, "xs2": xs2_c, "kb2": kb2_c,
                        "kapxs": kapxs_c, "bias_bc": bb_c,
                        "mmx": mmx, "cv": cv_c})
    return in_maps


def _run(x, weight, bias, trace=False, trace_kwargs=None):
    if "nc" not in _CACHE:
        _CACHE["nc"] = _build_nc()
    nc = _CACHE["nc"]
    in_maps = _prep_inputs(x, weight, bias)
    res = run_bass_kernel_spmd(nc, in_maps, list(range(N_CORES)), trace=trace,
                               **(trace_kwargs or {}))
    out = np.empty((B, N), dtype=np.float32)
    for c in range(N_CORES):
        b, h = divmod(c, NSH)
        out[b * BC:(b + 1) * BC, h * NC:(h + 1) * NC] = res.results[c]["out"]
    return out, res


def kernel(x, weight, bias):
    x = np.asarray(x, dtype=np.float32)
    weight = np.asarray(weight, dtype=np.float32)
    bias = np.asarray(bias, dtype=np.float32)
    out, _ = _run(x, weight, bias, trace=False)
    return out.astype(np.float32)
